# revision 11
# baseline (speedup 1.0000x reference)
"""Per-pixel adaptive 5x5 conv (KPN) for Trainium2, 8-core data parallel.

out[g,r,c] = sum_{i,j} core[g,5i+j,r,c] * frames_pad[g,r+i-2,c+j-2]
with g = flattened (B,N) = 16 image planes; 2 planes per NeuronCore.

Layout: partition p holds image rows 4p..4p+3 in the free dim
("rows-in-free"), so column taps (j) are free-dim offsets and row taps
(i) are merged on the TensorEngine with shift matrices:

  P_t[p,q,c] = W_t[p,q,c] * fin[p, q, joff_j + c]          (DVE fp16 2x)
  out[r]     = sum_t P_t[r + s_t],  s = i-2                (TensorE)

Host pre-shifts core rows by s so every P value needed outside [0,512)
lands on a zero-padded frame row -> contributes exactly 0; the 128x128
shift matrices (I / sub / super diagonal) truncate naturally at the
partition boundary. Per tap: 1 DVE mul + 4 matmuls (one per PSUM bank,
FD=512), accumulating in fp32 PSUM. GpSimd does no elementwise work
(it shares its SBUF port with the DVE; running both slows each ~4x).

DMA-stream discipline (the kernel sits at the DVE/HBM ridge, so every
HBM byte ahead of weights starves the DVE):
- only the parity-0 frame copy comes from HBM; the parity-1 copy (for
  odd-j taps' 4B alignment) is a 2-byte-shifted SBUF->SBUF DMA done as
  one flat 4KB run per partition (cols 0-1 of each row-block are never
  read), kept on the sync ring where it drains in-order at full rate;
- img1's frames are fetched mid-img0 where the weight-pool backlog
  absorbs the insertion; smat rides the idle scalar ring;
- PSUM is split into two 2-bank tiles per image so the final ScalarE
  and DVE drain copies run in parallel; outputs are fp16 stores with a
  host-side upcast.
"""

import os
import sys

import numpy as np

for _p in ("/opt/trn_rl_repo",):
    if _p not in sys.path and os.path.isdir(_p):
        sys.path.insert(0, _p)

K = 5
NCORES = 8
IMGS_PER_CORE = 2
H = W = 512
QR = 4                      # image rows per partition
FCOLS = 518
PAR_FREE = QR * FCOLS       # 2072 per parity copy
TAPS = K * K
O_FREE = QR * W             # 2048
HALF = O_FREE // 2          # 1024 (= 2 PSUM banks fp32)

# Tap (2,0) first (s=0 -> its matmuls initialize all PSUM banks with
# start=True). All even-j (parity-0) taps run before any odd-j tap so
# the on-chip parity-1 frame copy has ~19us of slack to land.
I_ORDER = (2, 0, 1, 3, 4)
TAP_LIST = tuple([(i, j) for j in (0, 2, 4) for i in I_ORDER]
                 + [(i, j) for j in (1, 3) for i in I_ORDER])

_compiled = {}
last_results = None  # BassKernelResults of the most recent run (for test.py)


def _build_nc():
    import concourse.bacc as bacc
    import concourse.mybir as mybir
    from concourse.tile import TileContext

    f16 = mybir.dt.float16
    f32 = mybir.dt.float32

    nc = bacc.Bacc(None, target_bir_lowering=False, debug=False)
    fin = nc.dram_tensor("fin", [IMGS_PER_CORE, 128, PAR_FREE], f16,
                         kind="ExternalInput")
    win = nc.dram_tensor("win", [IMGS_PER_CORE, TAPS, 128, O_FREE], f16,
                         kind="ExternalInput")
    smat = nc.dram_tensor("smat", [128, 3 * 128], f16, kind="ExternalInput")
    oout = nc.dram_tensor("oout", [IMGS_PER_CORE, 128, O_FREE], f16,
                          kind="ExternalOutput")

    with TileContext(nc) as tc:
        with (
            tc.tile_pool(name="cpool", bufs=1) as cpool,
            tc.tile_pool(name="fpool", bufs=4) as fpool,
            tc.tile_pool(name="wpool", bufs=16) as wpool,
            tc.tile_pool(name="ppool", bufs=6) as ppool,
            tc.tile_pool(name="opool", bufs=2) as opool,
            tc.psum_pool(name="pspool", bufs=2) as pspool,
        ):
            f_ts = [[None, None] for _ in range(IMGS_PER_CORE)]

            def fin_dma(img):
                # parity 0 from HBM; parity 1 = same data shifted one
                # column (2 bytes), flat 4142-elem run per partition.
                # Cols 0-1 of each q-block in the par1 view are never
                # read (odd-j taps have joff>=2), so ignoring q-block
                # boundaries is safe and keeps descriptors large.
                t0 = fpool.tile([128, PAR_FREE], f16, name=f"f{img}0",
                                tag=f"f{img}0")
                nc.sync.dma_start(out=t0[:], in_=fin[img])
                t1 = fpool.tile([128, PAR_FREE], f16, name=f"f{img}1",
                                tag=f"f{img}1")
                # SWDGE (gpsimd) path: its wait on f00's completion
                # happens on the otherwise-idle gpsimd queue instead of
                # blocking the in-order sync HWDGE ring that carries
                # the weight stream.
                nc.gpsimd.dma_start(out=t1[:, 1:PAR_FREE],
                                    in_=t0[:, 0:PAR_FREE - 1])
                f_ts[img] = [t0, t1]

            fin_dma(0)
            sm_t = cpool.tile([128, 3 * 128], f16)
            nc.scalar.dma_start(out=sm_t[:], in_=smat[:])
            sm = {"I": sm_t[:, 0:128], "P": sm_t[:, 128:256],
                  "M": sm_t[:, 256:384]}

            def fview(img, par):
                return f_ts[img][par][:].rearrange("p (q c) -> p q c", q=QR)

            # --- main tap stream ---
            for img in range(IMGS_PER_CORE):
                ps_lo = pspool.tile([128, HALF], f32, name="pslo", tag="pslo")
                ps_hi = pspool.tile([128, HALF], f32, name="pshi", tag="pshi")
                ps_half = (ps_lo, ps_hi)
                for t, (i, j) in enumerate(TAP_LIST):
                    s = i - 2
                    par = j & 1
                    joff = j + par
                    w_t = wpool.tile([128, O_FREE], f16, name="w", tag="w")
                    nc.sync.dma_start(out=w_t[:], in_=win[img, t])
                    if img == 0 and t == 14:
                        fin_dma(1)  # mid-stream, absorbed by wpool backlog
                    p_t = ppool.tile([128, O_FREE], f16, name="p", tag="p")
                    nc.vector.tensor_mul(
                        out=p_t[:].rearrange("p (q c) -> p q c", q=QR),
                        in0=w_t[:].rearrange("p (q c) -> p q c", q=QR),
                        in1=fview(img, par)[:, :, joff:joff + W])
                    for q in range(QR):
                        qs = q + s
                        if 0 <= qs < QR:
                            lhsT, rblk = sm["I"], qs
                        elif qs >= QR:
                            lhsT, rblk = sm["P"], qs - QR
                        else:
                            lhsT, rblk = sm["M"], qs + QR
                        nc.tensor.matmul(
                            out=ps_half[q // 2][:, (q % 2) * W:
                                                (q % 2 + 1) * W],
                            lhsT=lhsT,
                            rhs=p_t[:, rblk * W:(rblk + 1) * W],
                            start=(t == 0),
                            stop=(t == TAPS - 1))

                # drain: img0 on ScalarE (DVE is busy); final image on
                # ScalarE + DVE halves in parallel (independent PSUM
                # tiles and staging tiles -> no false serialization).
                last = img == IMGS_PER_CORE - 1
                o_lo = opool.tile([128, HALF], f16, name="olo", tag="olo")
                o_hi = opool.tile([128, HALF], f16, name="ohi", tag="ohi")
                nc.scalar.copy(out=o_lo[:], in_=ps_lo[:])
                nc.scalar.dma_start(out=oout[img][:, 0:HALF], in_=o_lo[:])
                if last:
                    nc.vector.tensor_copy(o_hi[:], ps_hi[:])
                    nc.sync.dma_start(out=oout[img][:, HALF:O_FREE],
                                      in_=o_hi[:])
                else:
                    nc.scalar.copy(out=o_hi[:], in_=ps_hi[:])
                    nc.scalar.dma_start(out=oout[img][:, HALF:O_FREE],
                                        in_=o_hi[:])
    nc.finalize()
    return nc


def _host_prep(frames, core):
    """Build per-core in_maps. frames [4,4,1,512,512] f32, core [4,4,25,1,512,512]."""
    G = NCORES * IMGS_PER_CORE  # 16
    F = np.ascontiguousarray(frames.reshape(G, H, W))
    C = core.reshape(G, TAPS, H, W)

    # parity-0 copy only: fin[p, q, cc] = Fc[4p+q, 1+cc]
    Fc = np.pad(F, ((0, 0), (0, 0), (3, 4))).astype(np.float16)  # [G,512,519]
    fin = np.ascontiguousarray(
        Fc[:, :, 1:1 + FCOLS].reshape(G, 128, QR * FCOLS))

    win = np.zeros((G, TAPS, H, W), np.float16)
    for t, (i, j) in enumerate(TAP_LIST):
        s = i - 2
        src = C[:, i * K + j]
        if s > 0:
            win[:, t, s:] = src[:, :H - s]
        elif s < 0:
            win[:, t, :s] = src[:, -s:]
        else:
            win[:, t] = src

    smat = np.concatenate([np.eye(128, dtype=np.float16),
                           np.eye(128, k=-1, dtype=np.float16),
                           np.eye(128, k=+1, dtype=np.float16)], axis=1)
    smat = np.ascontiguousarray(smat)

    win = win.reshape(G, TAPS, 128, O_FREE)
    in_maps = []
    for c in range(NCORES):
        g0 = c * IMGS_PER_CORE
        in_maps.append({
            "fin": np.ascontiguousarray(fin[g0:g0 + IMGS_PER_CORE]),
            "win": np.ascontiguousarray(win[g0:g0 + IMGS_PER_CORE]),
            "smat": smat,
        })
    return in_maps


def kernel(frames, core, bias):
    global last_results
    from concourse.bass_utils import run_bass_kernel_spmd

    frames = np.asarray(frames, dtype=np.float32)
    core = np.asarray(core, dtype=np.float32)

    if "nc" not in _compiled:
        _compiled["nc"] = _build_nc()
    nc = _compiled["nc"]

    in_maps = _host_prep(frames, core)
    trace = os.environ.get("KC_TRACE") == "1"
    tmpdir = os.environ.get("KC_TRACE_DIR") or None
    if tmpdir:
        os.makedirs(tmpdir, exist_ok=True)
    res = run_bass_kernel_spmd(nc, in_maps, list(range(NCORES)), trace=trace,
                               tmpdir=tmpdir)
    last_results = res

    G = NCORES * IMGS_PER_CORE
    out = np.empty((G, H, W), np.float32)
    for c in range(NCORES):
        o = res.results[c]["oout"]  # [2, 128, 2048] f16
        for img in range(IMGS_PER_CORE):
            out[c * IMGS_PER_CORE + img] = (
                o[img].reshape(H, W).astype(np.float32))
    return out.reshape(4, 4, H, W)


# revision 12
# speedup vs baseline: 1.1350x; 1.1350x over previous
"""Per-pixel adaptive 5x5 conv (KPN) for Trainium2, 8-core data parallel.

out[g,r,c] = sum_{i,j} core[g,5i+j,r,c] * frames_pad[g,r+i-2,c+j-2]
with g = flattened (B,N) = 16 image planes; 2 planes per NeuronCore.

Layout: partition p holds image rows 4p..4p+3 in the free dim
("rows-in-free"), so column taps (j) are free-dim offsets and row taps
(i) are merged on the TensorEngine with shift matrices:

  P_t[p,q,c] = W_t[p,q,c] * fin[p, q, joff_j + c]          (DVE fp16 2x)
  out[r]     = sum_t P_t[r + s_t],  s = i-2                (TensorE)

Host pre-shifts core rows by s so every P value needed outside [0,512)
lands on a zero-padded frame row -> contributes exactly 0; the 128x128
shift matrices (I / sub / super diagonal) truncate naturally at the
partition boundary. Per tap: 1 DVE mul + 4 matmuls (one per PSUM bank,
FD=512), accumulating in fp32 PSUM. GpSimd does no elementwise work
(it shares its SBUF port with the DVE; running both slows each ~4x).

DMA-stream discipline (the kernel sits at the DVE/HBM ridge, so every
HBM byte ahead of weights starves the DVE):
- only the parity-0 frame copy comes from HBM; the parity-1 copy (for
  odd-j taps' 4B alignment) is a 2-byte-shifted SBUF->SBUF DMA done as
  one flat 4KB run per partition (cols 0-1 of each row-block are never
  read), kept on the sync ring where it drains in-order at full rate;
- img1's frames are fetched mid-img0 where the weight-pool backlog
  absorbs the insertion; smat rides the idle scalar ring;
- PSUM is split into two 2-bank tiles per image so the final ScalarE
  and DVE drain copies run in parallel; outputs are fp16 stores with a
  host-side upcast.
"""

import os
import sys

import numpy as np

for _p in ("/opt/trn_rl_repo",):
    if _p not in sys.path and os.path.isdir(_p):
        sys.path.insert(0, _p)

K = 5
NCORES = 8
IMGS_PER_CORE = 2
H = W = 512
QR = 4                      # image rows per partition
FCOLS = 518
PAR_FREE = QR * FCOLS       # 2072 per parity copy
TAPS = K * K
O_FREE = QR * W             # 2048
HALF = O_FREE // 2          # 1024 (= 2 PSUM banks fp32)

# i=2 group (s=0) first so the first matmuls initialize all PSUM banks;
# within each group even-parity taps first so the par1 on-chip copy can
# finish a few taps late.
I_ORDER = (2, 0, 1, 3, 4)
J_ORDER = (0, 2, 4, 1, 3)
TAP_LIST = tuple((i, j) for i in I_ORDER for j in J_ORDER)

_compiled = {}
last_results = None  # BassKernelResults of the most recent run (for test.py)


def _build_nc():
    import concourse.bacc as bacc
    import concourse.mybir as mybir
    from concourse.tile import TileContext

    f16 = mybir.dt.float16
    f32 = mybir.dt.float32

    nc = bacc.Bacc(None, target_bir_lowering=False, debug=False)
    fin = nc.dram_tensor("fin", [IMGS_PER_CORE, 128, PAR_FREE], f16,
                         kind="ExternalInput")
    win = nc.dram_tensor("win", [IMGS_PER_CORE, TAPS, 128, O_FREE], f16,
                         kind="ExternalInput")
    smat = nc.dram_tensor("smat", [128, 3 * 128], f16, kind="ExternalInput")
    oout = nc.dram_tensor("oout", [IMGS_PER_CORE, 128, O_FREE], f16,
                          kind="ExternalOutput")

    with TileContext(nc) as tc:
        with (
            tc.tile_pool(name="cpool", bufs=1) as cpool,
            tc.tile_pool(name="fpool", bufs=4) as fpool,
            tc.tile_pool(name="wpool", bufs=14) as wpool,
            tc.tile_pool(name="ppool", bufs=6) as ppool,
            tc.tile_pool(name="opool", bufs=2) as opool,
            tc.psum_pool(name="pspool", bufs=2) as pspool,
        ):
            f_ts = [[None, None] for _ in range(IMGS_PER_CORE)]

            def fin_dma(img):
                # parity 0 from HBM; parity 1 = same data shifted one
                # column (2 bytes), flat 4142-elem run per partition.
                # Cols 0-1 of each q-block in the par1 view are never
                # read (odd-j taps have joff>=2), so ignoring q-block
                # boundaries is safe and keeps descriptors large.
                t0 = fpool.tile([128, PAR_FREE], f16, name=f"f{img}0",
                                tag=f"f{img}0")
                nc.sync.dma_start(out=t0[:], in_=fin[img])
                t1 = fpool.tile([128, PAR_FREE], f16, name=f"f{img}1",
                                tag=f"f{img}1")
                # SWDGE (gpsimd) path: its wait on f00's completion
                # happens on the otherwise-idle gpsimd queue instead of
                # blocking the in-order sync HWDGE ring that carries
                # the weight stream.
                nc.gpsimd.dma_start(out=t1[:, 1:PAR_FREE],
                                    in_=t0[:, 0:PAR_FREE - 1])
                f_ts[img] = [t0, t1]

            fin_dma(0)
            sm_t = cpool.tile([128, 3 * 128], f16)
            nc.scalar.dma_start(out=sm_t[:], in_=smat[:])
            sm = {"I": sm_t[:, 0:128], "P": sm_t[:, 128:256],
                  "M": sm_t[:, 256:384]}

            def fview(img, par):
                return f_ts[img][par][:].rearrange("p (q c) -> p q c", q=QR)

            # --- main tap stream ---
            for img in range(IMGS_PER_CORE):
                ps_lo = pspool.tile([128, HALF], f32, name="pslo", tag="pslo")
                ps_hi = pspool.tile([128, HALF], f32, name="pshi", tag="pshi")
                ps_half = (ps_lo, ps_hi)
                for t, (i, j) in enumerate(TAP_LIST):
                    s = i - 2
                    par = j & 1
                    joff = j + par
                    w_t = wpool.tile([128, O_FREE], f16, name="w", tag="w")
                    nc.sync.dma_start(out=w_t[:], in_=win[img, t])
                    if img == 0 and t == 14:
                        fin_dma(1)  # mid-stream, absorbed by wpool backlog
                    p_t = ppool.tile([128, O_FREE], f16, name="p", tag="p")
                    nc.vector.tensor_mul(
                        out=p_t[:].rearrange("p (q c) -> p q c", q=QR),
                        in0=w_t[:].rearrange("p (q c) -> p q c", q=QR),
                        in1=fview(img, par)[:, :, joff:joff + W])
                    for q in range(QR):
                        qs = q + s
                        if 0 <= qs < QR:
                            lhsT, rblk = sm["I"], qs
                        elif qs >= QR:
                            lhsT, rblk = sm["P"], qs - QR
                        else:
                            lhsT, rblk = sm["M"], qs + QR
                        nc.tensor.matmul(
                            out=ps_half[q // 2][:, (q % 2) * W:
                                                (q % 2 + 1) * W],
                            lhsT=lhsT,
                            rhs=p_t[:, rblk * W:(rblk + 1) * W],
                            start=(t == 0),
                            stop=(t == TAPS - 1))

                # drain: img0 on ScalarE (DVE is busy); final image on
                # ScalarE + DVE halves in parallel (independent PSUM
                # tiles and staging tiles -> no false serialization).
                last = img == IMGS_PER_CORE - 1
                o_lo = opool.tile([128, HALF], f16, name="olo", tag="olo")
                o_hi = opool.tile([128, HALF], f16, name="ohi", tag="ohi")
                nc.scalar.copy(out=o_lo[:], in_=ps_lo[:])
                nc.scalar.dma_start(out=oout[img][:, 0:HALF], in_=o_lo[:])
                if last:
                    nc.vector.tensor_copy(o_hi[:], ps_hi[:])
                    nc.sync.dma_start(out=oout[img][:, HALF:O_FREE],
                                      in_=o_hi[:])
                else:
                    nc.scalar.copy(out=o_hi[:], in_=ps_hi[:])
                    nc.scalar.dma_start(out=oout[img][:, HALF:O_FREE],
                                        in_=o_hi[:])
    nc.finalize()
    return nc


def _host_prep(frames, core):
    """Build per-core in_maps. frames [4,4,1,512,512] f32, core [4,4,25,1,512,512]."""
    G = NCORES * IMGS_PER_CORE  # 16
    F = np.ascontiguousarray(frames.reshape(G, H, W))
    C = core.reshape(G, TAPS, H, W)

    # parity-0 copy only: fin[p, q, cc] = Fc[4p+q, 1+cc]
    Fc = np.pad(F, ((0, 0), (0, 0), (3, 4))).astype(np.float16)  # [G,512,519]
    fin = np.ascontiguousarray(
        Fc[:, :, 1:1 + FCOLS].reshape(G, 128, QR * FCOLS))

    win = np.zeros((G, TAPS, H, W), np.float16)
    for t, (i, j) in enumerate(TAP_LIST):
        s = i - 2
        src = C[:, i * K + j]
        if s > 0:
            win[:, t, s:] = src[:, :H - s]
        elif s < 0:
            win[:, t, :s] = src[:, -s:]
        else:
            win[:, t] = src

    smat = np.concatenate([np.eye(128, dtype=np.float16),
                           np.eye(128, k=-1, dtype=np.float16),
                           np.eye(128, k=+1, dtype=np.float16)], axis=1)
    smat = np.ascontiguousarray(smat)

    win = win.reshape(G, TAPS, 128, O_FREE)
    in_maps = []
    for c in range(NCORES):
        g0 = c * IMGS_PER_CORE
        in_maps.append({
            "fin": np.ascontiguousarray(fin[g0:g0 + IMGS_PER_CORE]),
            "win": np.ascontiguousarray(win[g0:g0 + IMGS_PER_CORE]),
            "smat": smat,
        })
    return in_maps


def kernel(frames, core, bias):
    global last_results
    from concourse.bass_utils import run_bass_kernel_spmd

    frames = np.asarray(frames, dtype=np.float32)
    core = np.asarray(core, dtype=np.float32)

    if "nc" not in _compiled:
        _compiled["nc"] = _build_nc()
    nc = _compiled["nc"]

    in_maps = _host_prep(frames, core)
    trace = os.environ.get("KC_TRACE") == "1"
    tmpdir = os.environ.get("KC_TRACE_DIR") or None
    if tmpdir:
        os.makedirs(tmpdir, exist_ok=True)
    res = run_bass_kernel_spmd(nc, in_maps, list(range(NCORES)), trace=trace,
                               tmpdir=tmpdir)
    last_results = res

    G = NCORES * IMGS_PER_CORE
    out = np.empty((G, H, W), np.float32)
    for c in range(NCORES):
        o = res.results[c]["oout"]  # [2, 128, 2048] f16
        for img in range(IMGS_PER_CORE):
            out[c * IMGS_PER_CORE + img] = (
                o[img].reshape(H, W).astype(np.float32))
    return out.reshape(4, 4, H, W)
